# revision 30
# baseline (speedup 1.0000x reference)
"""Fused cosine-similarity kernel for Trainium2 (8 NeuronCores, data-parallel).

out[n, m] = (z_n / max(||z_n||, eps)) . (cm_m / max(||cm_m||, eps))

Sharding: z [32768, 512] split along n into 8 shards of 4096 rows; the
[1001, 512] centroid matrix is replicated; each core computes its own
[4096, 1001] output slab; host concatenates. No cross-core communication.

v2 (fp16 I/O): the 2e-2 rel-err budget is ~40x looser than fp16
rounding, so all bulk HBM traffic is fp16 — z is cast+transposed to
[d, n] on the host (4 MB/core), the centroid matrix is normalized,
transposed and padded on the host (1 MB replicated), and the output is
stored as fp16 (8 MB/core) then upcast on the host. 1/max(||z||,eps) is
computed on the host in fp32 (16 KB/core) and fused into the PSUM->SBUF
drain. That removes every PE transpose and all on-chip norm work: the
tensor engine runs only the 2.1 GMAC GEMM at 1 cycle/row fp16 — 32
tiles x 4004 streamed rows/pass, the bottleneck on HW (measured 43.2
us/pass steady, ~2.97 GHz effective PE clock; fp16 DMA is 12.2 MB/pass
~= 36.6 us at the ~350 GB/s roofline, vs 26.4 MB = 74.5 us for the fp32
baseline; cost model says 53.4 us at its 2.4 GHz PE). Per 128-row tile:
2 output chunks (512 | 489 cols) x 4 K-subtiles of PSUM-accumulated
matmuls; chunk drains split vector/scalar engine into ONE store per
tile (HWDGE configs cost ~630 ns each and 2 stores/tile made HWDGE a
near-second bottleneck at 46 us/pass).

Head/tail (9.7 us in-model): dummy matmuls on a zeroed tile cover the
PE p-state ramp while the head loads stream as small pieces (z tiles +
per-K-pair chunk-0 centroid halves first, chunk-1 block last) so the
first real matmul fires ~4 us in; the first 8 row-tiles run chunk-major
so chunk-0 work hides the chunk-1 centroid transfer; the final tile
stores per-chunk so the very last DMA is the small 489-col piece.
"""
import numpy as np

N_CORES = 8
N_FULL, D, M = 32768, 512, 1001
N_SHARD = N_FULL // N_CORES  # 4096
P = 128
KSUB = D // P  # 4
ROW_TILES = N_SHARD // P  # 32
EPS = 1e-8
# output column chunks: one PSUM bank holds 512 fp32 (cmT stays padded
# to 1008 for 2 KB DMA descriptor rows; fp16 matmul accepts the odd 489).
M_PAD = 1008
MM_N2 = 489  # streamed width of chunk 1 (cols 512:1001, exact odd width)

_CACHE = {}


def _legalize_waits(nc, cap=1):
    """Split multi-sem waits onto standalone EventSemaphore ops.

    The walrus build here encodes at most one sync-wait on several
    instruction encodings (fp32-weight matmuls fail at 2, Drain at 5).
    Sequential waits on the same engine are semantically identical.
    """
    import concourse.mybir as mybir
    ctr = 0
    for f in nc.m.functions:
        for blk in f.blocks:
            new_insts = []
            changed = False
            for inst in blk.instructions:
                si = getattr(inst, "sync_info", None)
                waits = list(si.on_wait) if si is not None else []
                if len(waits) > cap:
                    excess, keep = waits[:-cap], waits[-cap:]
                    for i in range(0, len(excess), cap):
                        w = mybir.InstEventSemaphore(
                            name=f"I-waitsplit-{ctr}", ins=[], outs=[])
                        ctr += 1
                        w.engine = inst.engine
                        w.sync_info = mybir.SyncInfo(
                            on_wait=excess[i:i + cap], on_update=[])
                        new_insts.append(w)
                    si.on_wait = keep
                    changed = True
                new_insts.append(inst)
            if changed:
                blk.instructions = new_insts
    return nc


def _build(reps=1, slab_bufs=3, osb_bufs=6, psmm_bufs=8, out_engines="va",
           taper_last=0, warm_mms=5, fine_head=1, load_q="s", store_qs="ss",
           head_qs="s", tail_q=None, tail_tiles=1, store_merge=1,
           slab_cols=512, head_slab1=1, c1_split=0, tail_split=1,
           head_kmajor=0, tail_fan=0):
    import concourse.bass as bass
    import concourse.mybir as mybir
    import concourse.tile as tile

    f16 = mybir.dt.float16
    f32 = mybir.dt.float32
    AF = mybir.ActivationFunctionType

    nc = bass.Bass()
    zt = nc.declare_dram_parameter("zt", [D, N_SHARD], f16, isOutput=False)
    cm = nc.declare_dram_parameter("cm", [D, M_PAD], f16, isOutput=False)
    rv = nc.declare_dram_parameter("rv", [P, ROW_TILES], f32, isOutput=False)
    out = nc.declare_dram_parameter("out", [N_SHARD, M], f16, isOutput=True)

    tiles_head = 4  # head region: first 4 row-tiles (512 z cols)

    with tile.TileContext(nc) as tc:
        qmap = {"s": nc.sync, "g": nc.gpsimd, "a": nc.scalar}
        LQ = qmap[load_q]
        SQ = [qmap[c] for c in store_qs]
        with (
            tc.tile_pool(name="singles", bufs=1) as singles,
            tc.tile_pool(name="zs", bufs=slab_bufs) as zsp,
            tc.tile_pool(name="osb", bufs=osb_bufs) as osb,
            tc.tile_pool(name="psmm", bufs=psmm_bufs, space="PSUM") as psmm,
        ):
            # ---- PE p-state warmup: dummy matmuls on a zeroed tile bridge
            # the head-DMA window so real matmuls start at full clock (the
            # ramp to 2.4 GHz takes 3 us of continuous PE execution).
            if warm_mms:
                warm = singles.tile([P, 512], f16)
                nc.vector.memset(warm[:].bitcast(f32), 0.0)
                pwarm = psmm.tile([P, 512], f32, tag="pm")
                for i in range(warm_mms):
                    nc.tensor.matmul(pwarm, warm[:, :P], warm,
                                     start=(i == 0), stop=(i == warm_mms - 1))

            cmT = singles.tile([P, KSUB, M_PAD], f16)
            rinv = singles.tile([P, ROW_TILES], f32)

            def load_range(n0, w):
                zs = zsp.tile([P, KSUB, w], f16, tag=f"zs{w}")
                LQ.dma_start(
                    zs, zt[:, n0:n0 + w].rearrange("(k p) n -> p k n", p=P))
                return zs

            # ---- head loads, finest pieces first so matmul 0 fires early.
            # HWDGE serializes DMA configs at ~630 ns apiece, so pieces are
            # kept few: z head tiles + chunk-0 centroid halves first, the
            # whole chunk-1 centroid block after.
            if fine_head:
                hq = [qmap[c] for c in head_qs]
                hseq = [0]

                def hdma(dst, src):
                    hq[hseq[0] % len(hq)].dma_start(dst, src)
                    hseq[0] += 1

                zh = [singles.tile([P, KSUB, P], f16, name=f"zh{t}")
                      for t in range(tiles_head)]

                def zh_load(t):
                    hdma(zh[t], zt[:, t * P:(t + 1) * P].rearrange(
                        "(k p) n -> p k n", p=P))

                zh_load(0)
                hdma(cmT[:, 0:2, 0:512],
                     cm[0:2 * P, 0:512].rearrange("(k p) m -> p k m", p=P))
                zh_load(1)
                hdma(cmT[:, 2:4, 0:512],
                     cm[2 * P:4 * P, 0:512].rearrange("(k p) m -> p k m", p=P))
                zh_load(2)
                zh_load(3)
                s1 = None
                if head_slab1:
                    # slab 1 joins the chunk-major head phase: its chunk-0
                    # work keeps the PE fed until the chunk-1 centroid
                    # block lands
                    s1 = zsp.tile([P, KSUB, 512], f16, tag="zs512")
                    hdma(s1, zt[:, 512:1024].rearrange("(k p) n -> p k n",
                                                       p=P))
                hdma(rinv, rv[:, :])
                if c1_split:
                    hdma(cmT[:, 0:2, 512:M_PAD],
                         cm[0:2 * P, 512:M_PAD].rearrange(
                             "(k p) m -> p k m", p=P))
                    hdma(cmT[:, 2:4, 512:M_PAD],
                         cm[2 * P:4 * P, 512:M_PAD].rearrange(
                             "(k p) m -> p k m", p=P))
                else:
                    hdma(cmT[:, :, 512:M_PAD], cm[:, 512:M_PAD].rearrange(
                        "(k p) m -> p k m", p=P))
                zh_src = zh
            else:
                slab0 = load_range(0, tiles_head * P)
                LQ.dma_start(cmT, cm[:, :].rearrange("(k p) m -> p k m", p=P))
                LQ.dma_start(rinv, rv[:, :])
                zh_src = None

            CHUNKS = [(0, 512, 512), (512, 1001, MM_N2)]

            def do_chunk(zsrc, tloc, gt, ci, ot, h):
                m0, m1, nwid = CHUNKS[ci]
                ri = rinv[:, gt:gt + 1]
                pm = psmm.tile([P, 512], f32, tag="pm")
                for k in range(KSUB):
                    nc.tensor.matmul(
                        pm[:, :nwid], zsrc[:, k, tloc * P:(tloc + 1) * P],
                        cmT[:, k, m0:m0 + nwid],
                        start=(k == 0), stop=(k == KSUB - 1))
                ncols = m1 - m0
                if out_engines[ci] == "a":
                    nc.scalar.activation(out=ot[:, h, m0:m1],
                                         in_=pm[:, :ncols],
                                         func=AF.Copy, scale=ri)
                else:
                    nc.vector.tensor_scalar_mul(ot[:, h, m0:m1],
                                                pm[:, :ncols], ri)

            def store_group(gt0, H, ot, sq=None):
                r0 = gt0 * P
                dst = out[r0:r0 + H * P, :].rearrange("(t p) m -> p t m", p=P)
                (sq or SQ[(gt0 // store_merge) % len(SQ)]).dma_start(dst, ot)

            def taper_tile(zsrc, tloc, gt, sq=None):
                # final tile: drain+store in half pieces per chunk so the
                # post-last-matmul chain is short
                ri = rinv[:, gt:gt + 1]
                r0 = gt * P
                for ci, (m0, m1, nwid) in enumerate(CHUNKS):
                    pm = psmm.tile([P, 512], f32, tag="pm")
                    for k in range(KSUB):
                        nc.tensor.matmul(
                            pm[:, :nwid], zsrc[:, k, tloc * P:(tloc + 1) * P],
                            cmT[:, k, m0:m0 + nwid],
                            start=(k == 0), stop=(k == KSUB - 1))
                    ncols = m1 - m0
                    hw_ = (ncols + 1) // 2
                    for piece, (p0, p1) in enumerate([(0, hw_), (hw_, ncols)]):
                        otp = osb.tile([P, 512], f16, tag="otp")
                        if (ci + piece) % 2 == 0:
                            nc.scalar.activation(out=otp[:, :p1 - p0],
                                                 in_=pm[:, p0:p1],
                                                 func=AF.Copy, scale=ri)
                        else:
                            nc.vector.tensor_scalar_mul(
                                otp[:, :p1 - p0], pm[:, p0:p1], ri)
                        (sq or SQ[piece % len(SQ)]).dma_start(
                            out[r0:r0 + P, m0 + p0:m0 + p1], otp[:, :p1 - p0])

            # ---- column-range job list: the first pass covers the head
            # tiles separately; later reps run the full shard so
            # slope(reps) isolates one steady pass.
            def make_ranges(start):
                rngs = []
                while start < N_SHARD:
                    w = min(slab_cols, N_SHARD - start)
                    rngs.append((start, w))
                    start += w
                return rngs

            head_ntiles = tiles_head + (4 if (head_slab1 and fine_head) else 0)
            jobs = make_ranges(head_ntiles * P)
            for _ in range(reps - 1):
                jobs += make_ranges(0)

            zs_next = load_range(*jobs[0]) if jobs else None

            # ---- head compute, chunk-major: only cmT[:, :, 0:512] and the
            # small z head tiles gate the first 16 matmuls
            hgroups = []
            t = 0
            while t < head_ntiles:
                H = min(store_merge, head_ntiles - t)
                ot = osb.tile([P, H, M], f16, tag=f"ot{H}")
                hgroups.append((t, H, ot))
                t += H

            def head_src(gt):
                if gt < tiles_head:
                    return zh_src[gt], 0
                return s1, gt - tiles_head

            if fine_head and head_kmajor:
                # k-major across the 4 zh tiles: k0/k1 matmuls of ALL
                # tiles run off the first centroid piece while the second
                # (k2/k3) piece streams, killing the cmB starvation gap
                hots = [osb.tile([P, 1, M], f16, tag="ot1", name=f"hot{i}")
                        for i in range(head_ntiles)]
                pms = []
                for gt in range(tiles_head):
                    pm = psmm.tile([P, 512], f32, tag="pm", name=f"hpm{gt}")
                    for k in (0, 1):
                        nc.tensor.matmul(pm, zh_src[gt][:, k, :],
                                         cmT[:, k, 0:512],
                                         start=(k == 0), stop=False)
                    pms.append(pm)
                for gt in range(tiles_head):
                    pm = pms[gt]
                    for k in (2, 3):
                        nc.tensor.matmul(pm, zh_src[gt][:, k, :],
                                         cmT[:, k, 0:512],
                                         start=False, stop=(k == 3))
                    ri = rinv[:, gt:gt + 1]
                    if out_engines[0] == "a":
                        nc.scalar.activation(out=hots[gt][:, 0, 0:512],
                                             in_=pm, func=AF.Copy, scale=ri)
                    else:
                        nc.vector.tensor_scalar_mul(hots[gt][:, 0, 0:512],
                                                    pm, ri)
                for gt in range(tiles_head, head_ntiles):
                    zsrc, tloc = head_src(gt)
                    do_chunk(zsrc, tloc, gt, 0, hots[gt], 0)
                for gt in range(head_ntiles):
                    zsrc, tloc = head_src(gt)
                    do_chunk(zsrc, tloc, gt, 1, hots[gt], 0)
                    store_group(gt, 1, hots[gt])
            elif fine_head:
                for ci in range(2):
                    for gt0, H, ot in hgroups:
                        for h in range(H):
                            zsrc, tloc = head_src(gt0 + h)
                            do_chunk(zsrc, tloc, gt0 + h, ci, ot, h)
                        if ci == 1:
                            store_group(gt0, H, ot)
            else:
                for gt0, H, ot in hgroups:
                    for h in range(H):
                        for ci in range(2):
                            do_chunk(slab0, gt0 + h, gt0 + h, ci, ot, h)
                    store_group(gt0, H, ot)

            # ---- steady ranges with one-ahead prefetch
            for i, (n0, w) in enumerate(jobs):
                zs_cur = zs_next
                zs_next = load_range(*jobs[i + 1]) if i + 1 < len(jobs) else None
                last_job = i == len(jobs) - 1
                ntiles = w // P
                t = 0
                while t < ntiles:
                    gt = n0 // P + t
                    H = min(store_merge, ntiles - t)
                    is_tail = last_job and (t + H >= ntiles - (tail_tiles - 1))
                    sq = qmap[tail_q] if (tail_q and is_tail) else None
                    if last_job and tail_split and t + H >= ntiles:
                        # final tile(s): per-chunk stores so the very last
                        # DMA is only the small 489-col piece
                        for h in range(H):
                            tl = t + h
                            gth = n0 // P + tl
                            fan = tail_fan and tl == ntiles - 1
                            otc = osb.tile([P, 1, M], f16, tag="ot1")
                            do_chunk(zs_cur, tl, gth, 0, otc, 0)
                            (sq or SQ[0]).dma_start(
                                out[gth * P:(gth + 1) * P, 0:512],
                                otc[:, 0, 0:512])
                            if not fan:
                                otd = osb.tile([P, 1, M], f16, tag="ot1")
                                do_chunk(zs_cur, tl, gth, 1, otd, 0)
                                (sq or SQ[1 % len(SQ)]).dma_start(
                                    out[gth * P:(gth + 1) * P, 512:M],
                                    otd[:, 0, 512:M])
                                continue
                            # fanned last chunk: half drains on both
                            # engines, stores on parallel queues
                            ri = rinv[:, gth:gth + 1]
                            pm = psmm.tile([P, 512], f32, tag="pm")
                            for k in range(KSUB):
                                nc.tensor.matmul(
                                    pm[:, :MM_N2],
                                    zs_cur[:, k, tl * P:(tl + 1) * P],
                                    cmT[:, k, 512:512 + MM_N2],
                                    start=(k == 0), stop=(k == KSUB - 1))
                            for piece, (p0, p1), eng, q in [
                                    (0, (0, 245), "v", nc.sync),
                                    (1, (245, 489), "a", nc.scalar)]:
                                otp = osb.tile([P, 512], f16, tag="otp")
                                if eng == "a":
                                    nc.scalar.activation(
                                        out=otp[:, :p1 - p0],
                                        in_=pm[:, p0:p1],
                                        func=AF.Copy, scale=ri)
                                else:
                                    nc.vector.tensor_scalar_mul(
                                        otp[:, :p1 - p0], pm[:, p0:p1], ri)
                                q.dma_start(
                                    out[gth * P:(gth + 1) * P,
                                        512 + p0:512 + p1],
                                    otp[:, :p1 - p0])
                        t += H
                        continue
                    if last_job and taper_last and t + H >= ntiles:
                        # final group: unmerged tiles, last one tapered
                        for h in range(H):
                            tl = t + h
                            gth = n0 // P + tl
                            if tl == ntiles - 1:
                                taper_tile(zs_cur, tl, gth, sq=sq)
                            else:
                                ot1 = osb.tile([P, 1, M], f16, tag="ot1")
                                for ci in range(2):
                                    do_chunk(zs_cur, tl, gth, ci, ot1, 0)
                                store_group(gth, 1, ot1, sq=sq)
                        t += H
                        continue
                    ot = osb.tile([P, H, M], f16, tag=f"ot{H}")
                    for h in range(H):
                        for ci in range(2):
                            do_chunk(zs_cur, t + h, gt + h, ci, ot, h)
                    store_group(gt, H, ot, sq=sq)
                    t += H

    _legalize_waits(nc)
    return nc


def prep_inputs(z, cluster_means):
    """Host-side shard + cast: returns the per-core input maps."""
    z = np.ascontiguousarray(z, dtype=np.float32)
    cmf = np.ascontiguousarray(cluster_means, dtype=np.float32)
    # fp32 norms with the reference's max(||.||, eps) semantics
    nrm = np.sqrt((cmf ** 2).sum(axis=1, keepdims=True, dtype=np.float32))
    cmn = (cmf / np.maximum(nrm, np.float32(EPS))).astype(np.float16)
    cmT = np.zeros((D, M_PAD), dtype=np.float16)
    cmT[:, :M] = cmn.T
    znrm = np.sqrt((z ** 2).sum(axis=1, dtype=np.float32))
    rinv = (np.float32(1.0) / np.maximum(znrm, np.float32(EPS))).astype(
        np.float32)
    zt_full = z.astype(np.float16).T  # [D, N_FULL]
    in_maps = []
    for c in range(N_CORES):
        c0 = c * N_SHARD
        in_maps.append({
            "zt": np.ascontiguousarray(zt_full[:, c0:c0 + N_SHARD]),
            "cm": cmT,
            # [p, t] layout: rinv for row t*128+p of this shard
            "rv": np.ascontiguousarray(
                rinv[c0:c0 + N_SHARD].reshape(ROW_TILES, P).T),
        })
    return in_maps


def kernel(z, cluster_means):
    from concourse.bass_utils import run_bass_kernel_spmd

    if "nc" not in _CACHE:
        _CACHE["nc"] = _build()
    nc = _CACHE["nc"]

    in_maps = prep_inputs(z, cluster_means)
    res = run_bass_kernel_spmd(nc, in_maps, core_ids=list(range(N_CORES)))
    return np.concatenate(
        [r["out"].astype(np.float32) for r in res.results], axis=0)
